# revision 9
# baseline (speedup 1.0000x reference)
"""Causal self-attention (B=4, T=2048, C=1024, H=16, D=64) on 8 trn2 NeuronCores.

Sharding: tensor-parallel over heads. Core g owns heads {2g, 2g+1}:
  - W_attn columns for those heads' q/k/v (128 cols each) -> per-core [1024, 384]
  - W_proj rows for those heads' channels -> per-core [128, 1024]
Each core computes a full [8192, 1024] partial of the output projection;
the host sums the 8 partials (the "all-reduce" of row-parallel W_proj).

Device layout notes:
  - Everything the PE touches is bf16 (1 cyc/row at any moving size, vs
    fp32r's 4x penalty under 256; also halves SBUF/HBM traffic). PSUM
    accumulation stays f32.
  - x is passed as X^T [C, B*T] so every matmul contracts over the partition
    dim. Attention uses the S^T = K @ Q^T formulation: S^T tiles are
    [k_tok, q_tok] so exp(S)*mask and the P^T @ V matmul need no on-chip
    transposes of P.
  - The softmax normalizer l[q] = sum_k P[k,q] comes from a ones column
    appended to V (stationary operand): one PSUM accumulation yields [y^T ; l].
  - 1/l: stage the PSUM l-row to a flat SBUF row (reciprocal_approx_fast
    mis-reads strided multi-bank PSUM APs), approx-reciprocal it, cast bf16,
    broadcast to 64 partitions with two K=1 matmuls.
  - Scheduling: S/exp run one step ahead of PV across the whole batch
    (including q-tile boundaries), and a filler queue interleaves QKV /
    V-transpose / projection work of other batches between attention steps.
    This keeps the PE continuously busy, which also holds it in the 2.4 GHz
    p-state. The Scalar queue carries only the exp stream (plus one l-row
    copy per q-tile) so the exp chain never waits behind filler copies.
"""

from collections import deque

import numpy as np

B, T, C, H, D = 4, 2048, 1024, 16, 64
NCORES = 8
BT = B * T                      # 8192
HPC = H // NCORES               # 2 heads per core
CPC = HPC * D                   # 128 channels per core
NC_CHUNKS = C // 128            # 8 contraction chunks of X^T
QW = 512                        # q-tile width (moving dim)
KW = 128                        # k-tile width (S^T partition dim)
NTT = T // QW                   # 4 token tiles per batch
NKT_B = T // KW                 # 16 k-tiles per batch

_CACHE = {}
LAST_RESULTS = None             # test harness reads exec_time_ns from here


def _build_bass():
    import concourse.bass as bass
    import concourse.mybir as mybir
    import concourse.tile as tile
    from concourse import bacc
    from concourse.masks import make_identity, make_upper_triangular

    f32 = mybir.dt.float32
    bf16 = mybir.dt.bfloat16
    Exp = mybir.ActivationFunctionType.Exp

    nc = bacc.Bacc()
    xt = nc.dram_tensor("xt", [C, BT], bf16, kind="ExternalInput")
    wg = nc.dram_tensor("wg", [C, 3 * CPC], bf16, kind="ExternalInput")
    bg = nc.dram_tensor("bg", [3 * CPC], f32, kind="ExternalInput")
    wp = nc.dram_tensor("wp", [CPC, C], bf16, kind="ExternalInput")
    outp = nc.dram_tensor("outp", [BT, C], bf16, kind="ExternalOutput")

    with tile.TileContext(nc) as tc:
        with (
            tc.tile_pool(name="const", bufs=1) as cpool,
            tc.tile_pool(name="sb", bufs=2) as sb,
            tc.tile_pool(name="ps", bufs=2, space="PSUM") as ps,
        ):
            # ---- weights (wg first: the first QKV matmuls need them) ----
            wg_sb = []
            for ci in range(NC_CHUNKS):
                wgt = cpool.tile([128, 3 * CPC], bf16, tag=f"wg{ci}")
                nc.sync.dma_start(out=wgt, in_=wg[ci * 128:(ci + 1) * 128, :])
                wg_sb.append(wgt)

            qkv = {}       # b -> (qt_sb, kt_sb, vt_sb)
            vaug = {}      # b -> [128, 16, 2, D+1] merged V^T tile
            xts = {}       # (b, tt) -> list of 8 x tiles
            fillers = deque()

            def emit_x_dma(b, tt):
                tok0 = b * T + tt * QW
                lst = []
                for ci in range(NC_CHUNKS):
                    xtile = sb.tile([128, QW], bf16, tag="xt", bufs=16,
                                    name="xtile")
                    nc.sync.dma_start(
                        out=xtile,
                        in_=xt[ci * 128:(ci + 1) * 128, tok0:tok0 + QW],
                    )
                    lst.append(xtile)
                xts[(b, tt)] = lst

            emit_x_dma(0, 0)

            # remaining constants / weights after the first x tiles
            wp_sb = cpool.tile([CPC, C], bf16, tag="wp")
            nc.sync.dma_start(out=wp_sb, in_=wp[:, :])
            bias_sb = []
            for grp in range(3):
                bt_ = cpool.tile([128, 1], f32, tag=f"bias{grp}")
                nc.sync.dma_start(
                    out=bt_,
                    in_=bg[grp * 128:(grp + 1) * 128].rearrange("(p o) -> p o", o=1),
                )
                bias_sb.append(bt_)

            scratch = cpool.tile([128, 128], f32, tag="scratch")
            make_identity(nc, scratch)
            identity = cpool.tile([128, 128], bf16, tag="ident")
            nc.vector.tensor_copy(identity, scratch)
            # mask[k, q] = 1.0 where q >= k else 0 (upper triangular incl diag)
            scratch2 = cpool.tile([128, 128], f32, tag="scratch2")
            make_upper_triangular(nc, scratch2, val=1.0, diag=True)
            mask = cpool.tile([128, 128], bf16, tag="mask")
            nc.vector.tensor_copy(mask, scratch2)
            # broadcast view of mask over the head axis (free-dim stride 0)
            mask2 = bass.AP(
                tensor=mask.tensor, offset=mask.offset,
                ap=[mask.ap[0], [0, 2], mask.ap[1]],
            )
            scratch3 = cpool.tile([128, 64], f32, tag="scratch3")
            nc.gpsimd.memset(scratch3, 1.0)
            ones_bf = cpool.tile([128, 64], bf16, tag="ones")
            nc.vector.tensor_copy(ones_bf, scratch3)

            def pump(n):
                """Emit up to n filler units (DMA-only units are free)."""
                while n > 0 and fillers:
                    kind, fn = fillers.popleft()
                    fn()
                    if kind != "dma":
                        n -= 1

            def emit_qkv_grp(b, tt, grp):
                """One q/k/v projection group for tokens [tt*QW, (tt+1)*QW)."""
                if b not in qkv:
                    qt_sb = sb.tile([128, T], bf16, tag="qt", name="qt_sb")
                    kt_sb = sb.tile([128, T], bf16, tag="kt", name="kt_sb")
                    vt_sb = sb.tile([128, T], bf16, tag="vt", name="vt_sb")
                    qkv[b] = (qt_sb, kt_sb, vt_sb)
                    va_all = sb.tile([128, NKT_B, 2, D + 1], bf16, tag="vaug",
                                     name="va_all")
                    nc.vector.tensor_copy(
                        va_all[:, :, :, D:D + 1],
                        ones_bf[:, 0:2 * NKT_B].rearrange(
                            "p (t h) -> p t h", h=2),
                    )
                    vaug[b] = va_all
                dest = qkv[b][grp]
                xl = xts[(b, tt)]
                pqkv = ps.tile([128, QW], f32, tag="mm", name="pqkv")
                for ci in range(NC_CHUNKS):
                    nc.tensor.matmul(
                        pqkv,
                        wg_sb[ci][:, grp * 128:(grp + 1) * 128],
                        xl[ci],
                        start=(ci == 0),
                        stop=(ci == NC_CHUNKS - 1),
                    )
                nc.vector.tensor_scalar_add(
                    out=dest[:, tt * QW:(tt + 1) * QW],
                    in0=pqkv,
                    scalar1=bias_sb[grp],
                )

            def emit_vtrans(b, kt):
                """V^T -> [V_A | 1], [V_B | 1] slices of the merged va tile."""
                vt_sb = qkv[b][2]
                ptr = ps.tile([128, 128], bf16, tag="mm",
                              padded_shape=[128, 1024], name="ptr")
                nc.tensor.transpose(
                    ptr, vt_sb[:, kt * KW:(kt + 1) * KW], identity
                )
                nc.vector.tensor_copy(
                    vaug[b][:, kt, :, 0:D],
                    ptr[:, 0:2 * D].rearrange("p (h x) -> p h x", x=D),
                )

            def emit_proj(b, qt, m, yt_sb):
                """Output projection for 128 tokens (m-th chunk of q-tile)."""
                tok0 = b * T
                osb = sb.tile([128, C], bf16, tag="osb", bufs=3, name="osb")
                for n in range(2):
                    pp = ps.tile([128, 512], f32, tag="mm", name="pp")
                    nc.tensor.matmul(
                        pp, yt_sb[:, m * 128:(m + 1) * 128],
                        wp_sb[:, n * 512:(n + 1) * 512],
                    )
                    nc.vector.tensor_copy(osb[:, n * 512:(n + 1) * 512], pp)
                row0 = tok0 + qt * QW + m * 128
                nc.gpsimd.dma_start(out=outp[row0:row0 + 128, :], in_=osb)

            def emit_norm(b, qt, y2):
                """y^T * broadcast(1/l) -> yt_sb [128, QW]; queue projection."""
                ystage = sb.tile([128, 2, QW], bf16, tag="ystage",
                                 name="ystage")
                nc.vector.tensor_copy(ystage[0:D, :, :], y2[0:D, :, :])
                # reciprocal_approx_fast mis-reads strided multi-bank PSUM
                # APs; stage l to a flat SBUF row first (Scalar engine).
                lrow = sb.tile([128, 2 * QW], f32, tag="lrow", name="lrow")
                nc.scalar.copy(
                    lrow[0:1, :].rearrange("p (h q) -> p h q", q=QW),
                    y2[D:D + 1, :, :],
                )
                rcpf = sb.tile([128, 2 * QW], f32, tag="rcpf", name="rcpf")
                nc.vector.reciprocal_approx_fast(
                    out=rcpf[0:1, :], in_=lrow[0:1, :]
                )
                rcr = sb.tile([128, 2 * QW], bf16, tag="rcr", name="rcr")
                nc.vector.tensor_copy(rcr[0:1, :], rcpf[0:1, :])
                bca = ps.tile([64, QW], f32, tag="mm", name="bca")
                bcb = ps.tile([64, QW], f32, tag="mm", name="bcb")
                nc.tensor.matmul(bca, ones_bf[0:1, 0:64], rcr[0:1, 0:QW])
                nc.tensor.matmul(bcb, ones_bf[0:1, 0:64], rcr[0:1, QW:2 * QW])
                yt_sb = sb.tile([128, QW], bf16, tag="yt", bufs=4,
                                name="yt_sb")
                nc.vector.tensor_mul(yt_sb[0:64, :], ystage[0:D, 0, :], bca)
                nc.vector.tensor_mul(yt_sb[64:128, :], ystage[0:D, 1, :], bcb)
                for m in range(QW // 128):
                    fillers.append(
                        ("proj",
                         lambda b=b, qt=qt, m=m, yt=yt_sb:
                             emit_proj(b, qt, m, yt))
                    )

            def attention_batch(b):
                qt_sb, kt_sb, _ = qkv[b]
                va_all = vaug[b]
                steps = []
                for qt in range(NTT):
                    nkt = (qt + 1) * (QW // KW)
                    kdiag = qt * (QW // KW)
                    for kt in range(nkt):
                        diag = kt >= kdiag
                        qoff = (kt - kdiag) * KW if diag else 0
                        steps.append((qt, kt, nkt, diag, qoff, QW - qoff))
                p_tiles = {}
                y2s = {}

                def emit_s_exp(i):
                    qt, kt, nkt, diag, qoff, w = steps[i]
                    qsl = slice(qt * QW + qoff, (qt + 1) * QW)
                    ksl = slice(kt * KW, (kt + 1) * KW)
                    st = ps.tile([128, 2, QW], f32, tag="st", name="st")
                    nc.tensor.matmul(
                        st[:, 0, 0:w], kt_sb[0:64, ksl], qt_sb[0:64, qsl]
                    )
                    nc.tensor.matmul(
                        st[:, 1, 0:w], kt_sb[64:128, ksl], qt_sb[64:128, qsl]
                    )
                    p = sb.tile([128, 2, QW], bf16, tag="p", bufs=4, name="p")
                    nc.scalar.activation(
                        p[:, :, 0:w], st[:, :, 0:w], Exp, scale=1.0 / np.sqrt(D)
                    )
                    if diag:
                        nc.vector.tensor_mul(p[:, :, 0:KW], p[:, :, 0:KW],
                                             mask2)
                    p_tiles[i] = p

                def emit_pv(i):
                    qt, kt, nkt, diag, qoff, w = steps[i]
                    if kt == 0:
                        y2s[qt] = ps.tile([D + 1, 2, QW], f32, tag="y",
                                          bufs=1, name="y2")
                    y2 = y2s[qt]
                    p = p_tiles.pop(i)
                    nc.tensor.matmul(
                        y2[:, 0, qoff:QW], va_all[:, kt, 0, :], p[:, 0, 0:w],
                        start=(kt == 0), stop=(kt == nkt - 1),
                    )
                    nc.tensor.matmul(
                        y2[:, 1, qoff:QW], va_all[:, kt, 1, :], p[:, 1, 0:w],
                        start=(kt == 0), stop=(kt == nkt - 1),
                    )

                emit_s_exp(0)
                for i, (qt, kt, nkt, diag, qoff, w) in enumerate(steps):
                    if i + 1 < len(steps):
                        emit_s_exp(i + 1)
                    pump(1)
                    emit_pv(i)
                    if kt == nkt - 1:
                        emit_norm(b, qt, y2s.pop(qt))

            def stage_batch_fillers(b):
                """Queue QKV + V-transpose work for batch b as filler units."""
                fillers.append(("dma", lambda b=b: emit_x_dma(b, 0)))
                fillers.append(("dma", lambda b=b: emit_x_dma(b, 1)))
                for tt in range(NTT):
                    if tt + 2 < NTT:
                        fillers.append(
                            ("dma", lambda b=b, tt=tt: emit_x_dma(b, tt + 2))
                        )
                    for grp in range(3):
                        fillers.append(
                            ("qkv",
                             lambda b=b, tt=tt, g=grp: emit_qkv_grp(b, tt, g))
                        )
                    for kt in range(tt * 4, tt * 4 + 4):
                        fillers.append(
                            ("vt", lambda b=b, kt=kt: emit_vtrans(b, kt))
                        )

            # ---- schedule ----
            # prologue: batch 0's QKV + V transposes emitted directly
            for tt in range(NTT):
                if tt > 0:
                    emit_x_dma(0, tt)
                for grp in range(3):
                    emit_qkv_grp(0, tt, grp)
                for kt in range(tt * 4, tt * 4 + 4):
                    emit_vtrans(0, kt)
            for b in range(B):
                if b + 1 < B:
                    stage_batch_fillers(b + 1)
                attention_batch(b)
            while fillers:
                _, fn = fillers.popleft()
                fn()

    nc.finalize()
    return nc


def _get_nc():
    if "nc" not in _CACHE:
        _CACHE["nc"] = _build_bass()
    return _CACHE["nc"]


def kernel(x, W_attn, b_attn, W_proj, b_proj):
    global LAST_RESULTS
    import ml_dtypes
    from concourse import bass_utils

    bf = ml_dtypes.bfloat16
    x = np.asarray(x, dtype=np.float32)
    W_attn = np.asarray(W_attn, dtype=np.float32)
    b_attn = np.asarray(b_attn, dtype=np.float32)
    W_proj = np.asarray(W_proj, dtype=np.float32)
    b_proj = np.asarray(b_proj, dtype=np.float32)

    xt_full = np.ascontiguousarray(x.reshape(BT, C).T.astype(bf))  # [C, B*T]

    in_maps = []
    for g in range(NCORES):
        cols = slice(g * CPC, (g + 1) * CPC)
        wg_g = np.ascontiguousarray(np.concatenate(
            [W_attn[:, cols], W_attn[:, C:][:, cols], W_attn[:, 2 * C:][:, cols]],
            axis=1,
        ).astype(bf))
        bg_g = np.ascontiguousarray(np.concatenate(
            [b_attn[cols], b_attn[C:][cols], b_attn[2 * C:][cols]]
        ))
        wp_g = np.ascontiguousarray(W_proj[cols, :].astype(bf))
        in_maps.append({"xt": xt_full, "wg": wg_g, "bg": bg_g, "wp": wp_g})

    nc = _get_nc()
    res = bass_utils.run_bass_kernel_spmd(nc, in_maps, core_ids=list(range(NCORES)))
    LAST_RESULTS = res

    acc = np.zeros((BT, C), dtype=np.float64)
    for r_ in res.results:
        acc += np.asarray(r_["outp"], dtype=np.float64)
    acc += b_proj
    return acc.astype(np.float32).reshape(B, T, C)
